# revision 12
# baseline (speedup 1.0000x reference)
"""MoE FFN (top-2 of 8 experts, d_model=1024, d_hid=4096) on 8 TRN2 NeuronCores.

Expert-parallel with hidden-dim pairing: the host router (0.006% of FLOPs)
produces the data-dependent dispatch; experts are paired (largest count
with smallest) and each pair is assigned 2 cores, each computing BOTH
experts of the pair over half (2048) of the hidden dim.  The host sums the
two partial outputs per expert (it already does a weighted combine).  This
balances per-core columns to CA+CB = max(small)+max(big) over pairs
(~2092) instead of 2*max(all) (~2144).

Device program per core (all matmuls bf16, f32 PSUM accumulation), per
segment s in {A, B} (one expert each, hidden rows h*2048..h*2048+2048):
  Phase 1:  hT[2048, cs] = gelu(w1h[s]^T @ xg[:, seg_s] + b1h[s])
  Phase 2:  outT[:, seg_s] = w2h[s]^T @ hT   (partial over the hidden half)
Host: out[token] += top_w * (outT_half0 + outT_half1)[:, col].T; + gates@b2.

Perf notes (measured on this hw):
  - The PE sustains only ~1.9-2.0 GHz with all 8 cores busy (power
    envelope; 2.4 GHz solo).  The kernel is within ~5% of that sustained
    roofline; remaining overhead is ~18ns per weight-group intercept.
  - Both weight matrices are SBUF-resident (16MB); hT is staged per
    segment so everything fits.  Steady-state DMA is xg in + outT (bf16)
    out only.
  - The Tile scheduler attaches a semaphore increment to EVERY matmul;
    _strip_pe_incs drops the unreferenced ones and remaps thresholds.
  - Legalization emits one LDWEIGHTS per matmul; redundant reloads of the
    same stationary tile are stripped post-compile (neutral on HW since
    LDW is hidden, but shrinks the instruction stream).
"""

import os
import sys

import numpy as np
import ml_dtypes

try:
    import concourse  # noqa: F401
except ImportError:  # fall back to the in-container repo checkout
    for _p in ("/opt/trn_rl_repo", os.path.expanduser("~/.axon_site/_ro/trn_rl_repo")):
        if os.path.isdir(_p) and _p not in sys.path:
            sys.path.insert(0, _p)

import concourse.mybir as mybir
import concourse.tile as tile
from concourse import bacc
from concourse.bass_utils import run_bass_kernel_spmd

D_MODEL, D_HID, N_EXPERTS, TOP_K = 1024, 4096, 8, 2
N_CORES = 8
P = 128
HO2 = D_HID // 2 // P  # 16 h-blocks per hidden half
DO = D_MODEL // P      # 8 d-blocks

BF16 = mybir.dt.bfloat16
F32 = mybir.dt.float32

_program_cache: dict[tuple, object] = {}
_weights_cache: dict = {}


# --------------------------------------------------------------------------
# Semaphore-increment stripping (see module docstring).
# --------------------------------------------------------------------------
def _strip_pe_incs(nc):
    fn = nc.m.functions[0]
    blocks = list(fn.blocks)

    def is_pe(name):
        return name.startswith("PE")

    block_incs = {}
    for bi, b in enumerate(blocks):
        incs = []
        for i in b.instructions:
            si = i.sync_info
            if si is None:
                continue
            for u in si.on_update:
                if (u.sync_type == "semaphore" and is_pe(u.ant_name)
                        and u.update_mode == "sem-inc"):
                    incs.append(i)
        if incs:
            block_incs[bi] = incs
    if len(block_incs) != 1:
        return 0
    (_, incs), = block_incs.items()
    n = len(incs)

    refs, wait_sites, upd_sites = set(), [], []
    for b in blocks:
        for i in b.instructions:
            si = i.sync_info
            if si is None:
                continue
            for w in si.on_wait:
                if (w.sync_type == "semaphore" and is_pe(w.ant_name)
                        and w.wait_mode in ("sem-ge-imm", "sem-eq-imm")
                        and w.wait_value is not None and 1 <= w.wait_value <= n):
                    refs.add(w.wait_value)
                    wait_sites.append((i, w))
            for u in si.on_update:
                if (u.sync_type == "semaphore" and is_pe(u.ant_name)
                        and u.update_mode != "sem-inc"
                        and u.update_value is not None
                        and 1 <= u.update_value <= n):
                    upd_sites.append((i, u))

    refs.add(n)
    keep = [False] * (n + 1)
    for v in refs:
        keep[v] = True
    for pos, inst in enumerate(incs, start=1):
        if "Matmult" not in type(inst).__name__:
            keep[pos] = True
    new_at, cnt = [0] * (n + 1), 0
    for p in range(1, n + 1):
        if keep[p]:
            cnt += 1
        new_at[p] = cnt

    dropped = 0
    for pos, inst in enumerate(incs, start=1):
        if keep[pos]:
            continue
        si = inst.sync_info
        inst.sync_info = mybir.SyncInfo(
            on_wait=list(si.on_wait),
            on_update=[u for u in si.on_update
                       if not (u.sync_type == "semaphore" and is_pe(u.ant_name)
                               and u.update_mode == "sem-inc")],
        )
        dropped += 1

    for wi, w in wait_sites:
        v, nv = w.wait_value, new_at[w.wait_value]
        if nv == v:
            continue
        si = wi.sync_info
        wi.sync_info = mybir.SyncInfo(
            on_wait=[
                mybir.SyncWait(sync_type=x.sync_type, id=x.id, ant_name=x.ant_name,
                               wait_mode=x.wait_mode, wait_value=nv,
                               wait_reg=x.wait_reg)
                if (x.sync_type == "semaphore" and is_pe(x.ant_name)
                    and x.wait_mode == w.wait_mode and x.wait_value == v)
                else x
                for x in si.on_wait
            ],
            on_update=list(si.on_update),
        )

    for ui, u in upd_sites:
        v, nv = u.update_value, new_at[u.update_value]
        if nv == v:
            continue
        si = ui.sync_info
        ui.sync_info = mybir.SyncInfo(
            on_wait=list(si.on_wait),
            on_update=[
                mybir.SyncUpdate(sync_type=x.sync_type, id=x.id,
                                 ant_name=x.ant_name, update_mode=x.update_mode,
                                 update_value=nv, update_reg=x.update_reg)
                if (x.sync_type == "semaphore" and is_pe(x.ant_name)
                    and x.update_mode == u.update_mode and x.update_value == v)
                else x
                for x in si.on_update
            ],
        )
    return dropped


# --------------------------------------------------------------------------
# Redundant-LDWEIGHTS stripping: legalization inserts one InstLdweights per
# InstMatmult even when consecutive matmuls use the identical stationary
# tile.  The array keeps its weights until the next LDWEIGHTS, so a reload
# of the exact same AP is a no-op -- drop it.  Tracker resets per block (a
# loop body's first LDW follows the previous iteration's LAST LDW, so
# first occurrences are always kept).  LDWs carrying sync are kept.
# --------------------------------------------------------------------------
def _strip_redundant_ldweights(nc):
    removed = 0
    for b in nc.m.functions[0].blocks:
        prev_key = None
        keep = []
        for inst in b.instructions:
            if isinstance(inst, mybir.InstLdweights):
                key = str(inst.ins[0])
                si = inst.sync_info
                has_sync = si is not None and (
                    len(si.on_wait) > 0 or len(si.on_update) > 0
                )
                if key == prev_key and not has_sync:
                    removed += 1
                    continue
                prev_key = key
            keep.append(inst)
        if removed:
            b.instructions[:] = keep
    return removed


def _n_tiles(C: int):
    """Split C columns into PSUM-bank-sized (<=512 f32) n-tiles."""
    tiles, off = [], 0
    while off < C:
        w = min(512, C - off)
        tiles.append((off, w))
        off += w
    return tiles


def _build_program(CA: int, CB: int = 0, repeat: int = 1, staggered: bool = False,
                   sem_strip: bool = True):
    """Two-segment program: segment A (cols [0,CA), expert A, hidden half)
    then segment B (cols [CA,CA+CB), expert B).  CB=0 -> single segment."""
    from concourse.bass import ds

    if staggered:
        # The staggered-reset sem protocol is incompatible with the PE-inc
        # strip (measured: deadlock); staggered alone wins anyway.
        sem_strip = False

    segments = [(0, 0, CA)]
    if CB:
        segments.append((1, CA, CB))
    C = CA + CB
    NSEG = len(segments)
    hmax = max(cw for _, _, cw in segments)

    nc = bacc.Bacc(
        "TRN2",
        target_bir_lowering=False,
        debug=False,
        num_devices=N_CORES,
    )
    xgT = nc.dram_tensor("xgT", [D_MODEL, C], BF16, kind="ExternalInput").ap()
    # host-packed: w1p[ki, s, m, ko, f] = w1half_s[ko*128+ki, m*128+f]
    w1 = nc.dram_tensor("w1", [P, NSEG, HO2, DO, P], BF16,
                        kind="ExternalInput").ap()
    # host-packed: w2p[ki, s, ko, d] = w2half_s[ko*128+ki, d]
    w2 = nc.dram_tensor("w2", [P, NSEG, HO2, D_MODEL], BF16,
                        kind="ExternalInput").ap()
    b1 = nc.dram_tensor("b1", [P, NSEG, HO2], F32, kind="ExternalInput").ap()
    outT = nc.dram_tensor("outT", [D_MODEL, C], BF16, kind="ExternalOutput").ap()

    xgT_r = xgT.rearrange("(po pi) f -> pi po f", pi=P)    # [128, 8, C]
    outT_r = outT.rearrange("(po pi) f -> pi po f", pi=P)  # [128, 8, C]

    with tile.TileContext(nc) as tc:
        with (
            tc.tile_pool(name="const", bufs=1) as const_pool,
            tc.tile_pool(name="ht_res", bufs=1) as ht_pool,
            tc.tile_pool(name="xg_res", bufs=1) as xg_pool,
            tc.tile_pool(name="w1_res", bufs=1) as w1_pool,
            tc.tile_pool(name="w2_res", bufs=1) as w2_pool,
            tc.tile_pool(name="p2_out", bufs=3) as out_pool,
            tc.tile_pool(name="psum", bufs=2, space="PSUM") as pp,
        ):
            b1_sb = const_pool.tile([P, NSEG, HO2], F32)
            nc.sync.dma_start(b1_sb[:], b1[:])

            # hT staged per segment so both resident weight matrices fit.
            hT_sb = ht_pool.tile([P, HO2, hmax], BF16)
            xg_sb = xg_pool.tile([P, DO, C], BF16)
            w1_sb = w1_pool.tile([P, NSEG, HO2, DO, P], BF16)
            w2_sb = w2_pool.tile([P, NSEG, HO2, D_MODEL], BF16)

            # Weights are iteration-invariant: prefetch once, outside the
            # loop body, in chunks on the gpsimd SWDGE queue.
            for s in range(NSEG):
                for j in range(HO2):
                    nc.gpsimd.dma_start(w1_sb[:, s, ds(j, 1)], w1[:, s, ds(j, 1)])
                    nc.gpsimd.dma_start(w2_sb[:, s, ds(j, 1)], w2[:, s, ds(j, 1)])

            def body():
                nc.sync.dma_start(xg_sb[:, 0], xgT_r[:, 0])
                nc.sync.dma_start(xg_sb[:, 1], xgT_r[:, 1])
                for j in range(2, DO):
                    nc.scalar.dma_start(xg_sb[:, j], xgT_r[:, j])

                for s, coff, cw in segments:
                    NT = _n_tiles(cw)
                    # ---------- Phase 1 (segment s, hidden half) ----------
                    for m in range(HO2):
                        psums = [
                            pp.tile([P, w], F32, name=f"p1_{s}_{m}_{i}",
                                    tag=f"ps_{i}")
                            for i, (toff, w) in enumerate(NT)
                        ]
                        # i-outer/k-inner: each PSUM bank takes its full
                        # 8-MM accumulation run back-to-back (no bank
                        # cycling between consecutive matmuls); the extra
                        # per-k weight reloads are free (LDW hides).
                        for i, (toff, w) in enumerate(NT):
                            for k in range(DO):
                                nc.tensor.matmul(
                                    psums[i][:],
                                    w1_sb[:, s, m, k],
                                    xg_sb[:, k, ds(coff + toff, w)],
                                    start=(k == 0),
                                    stop=(k == DO - 1),
                                )
                        for i, (toff, w) in enumerate(NT):
                            nc.scalar.activation(
                                hT_sb[:, m, ds(toff, w)],
                                psums[i][:],
                                mybir.ActivationFunctionType.Gelu,
                                bias=b1_sb[:, s, ds(m, 1)],
                            )

                    # ---------- Phase 2 (segment s, partial out) ----------
                    for m in range(DO):
                        psums = [
                            pp.tile([P, w], F32, name=f"p2_{s}_{m}_{i}",
                                    tag=f"ps_{i}")
                            for i, (toff, w) in enumerate(NT)
                        ]
                        for i, (toff, w) in enumerate(NT):
                            for k in range(HO2):
                                nc.tensor.matmul(
                                    psums[i][:],
                                    w2_sb[:, s, k, ds(m * P, P)],
                                    hT_sb[:, k, ds(toff, w)],
                                    start=(k == 0),
                                    stop=(k == HO2 - 1),
                                )
                        ot = out_pool.tile([P, cw], BF16, tag="ot")
                        for i, (toff, w) in enumerate(NT):
                            nc.vector.tensor_copy(
                                out=ot[:, ds(toff, w)], in_=psums[i][:])
                        nc.sync.dma_start(outT_r[:, m, ds(coff, cw)], ot[:])

            if repeat > 1:
                hints = tuple(
                    getattr(mybir.EngineType, e)
                    for e in ("PE", "SP", "Activation", "DVE", "Pool")
                    if hasattr(mybir.EngineType, e)
                )
                with tc.For_i(
                    0, repeat, 1, hint_engines=hints, staggered_reset=staggered
                ):
                    body()
            else:
                body()

    if sem_strip:
        _strip_pe_incs(nc)
    nc.compile()
    _strip_redundant_ldweights(nc)
    return nc


def _route(x, gate_w):
    """Host router: softmax + top-2 + renormalize. Returns dispatch lists."""
    xf = np.ascontiguousarray(np.asarray(x, dtype=np.float32)).reshape(-1, D_MODEL)
    n_tok = xf.shape[0]
    gw = np.asarray(gate_w, dtype=np.float32)
    logits = xf @ gw.T  # [N, E]
    m = logits.max(axis=-1, keepdims=True)
    e = np.exp(logits - m, dtype=np.float32)
    scores = e / e.sum(axis=-1, keepdims=True)
    top_i = np.argpartition(-scores, TOP_K - 1, axis=-1)[:, :TOP_K]  # [N, K]
    top_w = np.take_along_axis(scores, top_i, axis=-1)
    top_w = top_w / top_w.sum(axis=-1, keepdims=True)
    idx_per_e, w_per_e = [], []
    for ex in range(N_EXPERTS):
        tok, slot = np.nonzero(top_i == ex)
        idx_per_e.append(tok)
        w_per_e.append(top_w[tok, slot])
    return xf, n_tok, scores, idx_per_e, w_per_e


def _pairing(idx_per_e):
    """Pair experts (largest count with smallest).  Returns
    (pairs, CA, CB): pairs[p] = (eA, eB) with count(eA) <= count(eB);
    CA/CB = padded max over pairs of the A/B segment widths."""
    counts = np.array([len(ix) for ix in idx_per_e])
    order = np.argsort(counts, kind="stable")
    pairs = [(int(order[p]), int(order[N_EXPERTS - 1 - p]))
             for p in range(N_EXPERTS // 2)]
    pad = lambda c: max(16, (int(c) + 3) // 4 * 4)  # noqa: E731
    CA = max(pad(counts[a]) for a, _ in pairs)
    CB = max(pad(counts[b]) for _, b in pairs)
    return pairs, int(CA), int(CB)


def _pack_weights(w1, w2, b1, pairs):
    """Cast + pre-pack per-core weight halves for the device layout."""
    wkey = (id(w1), id(w2), tuple(x for p in pairs for x in p))
    cached = _weights_cache.get(wkey)
    if cached is None:
        w1f = np.asarray(w1, dtype=np.float32)
        w2f = np.asarray(w2, dtype=np.float32)
        b1f = np.asarray(b1, dtype=np.float32).reshape(N_EXPERTS, D_HID)
        H2 = D_HID // 2
        w1p, w2p, b1p = [], [], []
        for p, (eA, eB) in enumerate(pairs):
            for h in range(2):
                r0, r1 = h * H2, (h + 1) * H2
                w1c = np.empty((P, 2, HO2, DO, P), dtype=ml_dtypes.bfloat16)
                w2c = np.empty((P, 2, HO2, D_MODEL), dtype=ml_dtypes.bfloat16)
                b1c = np.empty((P, 2, HO2), dtype=np.float32)
                for s, e in enumerate((eA, eB)):
                    a = w1f[e][:, r0:r1].astype(ml_dtypes.bfloat16)  # [1024, 2048]
                    w1c[:, s] = a.reshape(DO, P, HO2, P).transpose(1, 2, 0, 3)
                    b = w2f[e][r0:r1].astype(ml_dtypes.bfloat16)     # [2048, 1024]
                    w2c[:, s] = b.reshape(HO2, P, D_MODEL).transpose(1, 0, 2)
                    b1c[:, s] = b1f[e][r0:r1].reshape(HO2, P).T
                w1p.append(np.ascontiguousarray(w1c))
                w2p.append(np.ascontiguousarray(w2c))
                b1p.append(np.ascontiguousarray(b1c))
        cached = (w1p, w2p, b1p)
        _weights_cache.clear()
        _weights_cache[wkey] = cached
    return cached


def _make_in_maps(xf, idx_per_e, pairs, CA, CB, w1p, w2p, b1p):
    in_maps = []
    for p, (eA, eB) in enumerate(pairs):
        xgT = np.zeros((D_MODEL, CA + CB), dtype=ml_dtypes.bfloat16)
        ixA, ixB = idx_per_e[eA], idx_per_e[eB]
        xgT[:, :len(ixA)] = xf[ixA].T.astype(ml_dtypes.bfloat16)
        xgT[:, CA:CA + len(ixB)] = xf[ixB].T.astype(ml_dtypes.bfloat16)
        for h in range(2):
            core = 2 * p + h
            in_maps.append({
                "xgT": xgT,
                "w1": w1p[core],
                "w2": w2p[core],
                "b1": b1p[core],
            })
    return in_maps


def _run_device(x, gate_w, w1, b1, w2, b2):
    xf, n_tok, _scores, idx_per_e, w_per_e = _route(x, gate_w)
    pairs, CA, CB = _pairing(idx_per_e)

    key = (CA, CB)
    if key not in _program_cache:
        _program_cache[key] = _build_program(CA, CB)
    nc = _program_cache[key]

    w1p, w2p, b1p = _pack_weights(w1, w2, b1, pairs)
    in_maps = _make_in_maps(xf, idx_per_e, pairs, CA, CB, w1p, w2p, b1p)

    res = run_bass_kernel_spmd(nc, in_maps, core_ids=list(range(N_CORES)))

    out_full = np.zeros((n_tok, D_MODEL), dtype=np.float32)
    for p, (eA, eB) in enumerate(pairs):
        acc = (np.asarray(res.results[2 * p]["outT"]).astype(np.float32)
               + np.asarray(res.results[2 * p + 1]["outT"]).astype(np.float32))
        for s, e in enumerate((eA, eB)):
            ix = idx_per_e[e]
            off = 0 if s == 0 else CA
            out_full[ix] += w_per_e[e][:, None] * acc[:, off:off + len(ix)].T
    gates = np.zeros((n_tok, N_EXPERTS), dtype=np.float32)
    for ex in range(N_EXPERTS):
        gates[idx_per_e[ex], ex] = w_per_e[ex]
    out_full += gates @ np.asarray(b2, dtype=np.float32).reshape(N_EXPERTS, D_MODEL)
    return out_full, res


def kernel(x, gate_w, w1, b1, w2, b2):
    out_full, _res = _run_device(x, gate_w, w1, b1, w2, b2)
    B, T, _ = np.asarray(x).shape
    return out_full.reshape(B, T, D_MODEL)


def _bench_maps(inputs):
    """(in_maps, (CA, CB)) for test.py's repeat-loop timing harness."""
    xf, n_tok, _s, idx_per_e, w_per_e = _route(inputs["x"], inputs["gate_w"])
    pairs, CA, CB = _pairing(idx_per_e)
    w1p, w2p, b1p = _pack_weights(inputs["w1"], inputs["w2"], inputs["b1"], pairs)
    in_maps = _make_in_maps(xf, idx_per_e, pairs, CA, CB, w1p, w2p, b1p)
    return in_maps, (CA, CB)


# revision 17
# speedup vs baseline: 1.0101x; 1.0101x over previous
"""MoE FFN (top-2 of 8 experts, d_model=1024, d_hid=4096) on 8 TRN2 NeuronCores.

Expert-parallel with hidden-dim pairing: the host router (0.006% of FLOPs)
produces the data-dependent dispatch; experts are paired (largest count
with smallest) and each pair is assigned 2 cores, each computing BOTH
experts of the pair over half (2048) of the hidden dim.  The host sums the
two partial outputs per expert (it already does a weighted combine).  This
balances per-core columns to CA+CB = max(small)+max(big) over pairs
(~2092) instead of 2*max(all) (~2144).

Device program per core (all matmuls bf16, f32 PSUM accumulation), per
segment s in {A, B} (one expert each, hidden rows h*2048..h*2048+2048):
  Phase 1:  hT[2048, cs] = gelu(w1h[s]^T @ xg[:, seg_s] + b1h[s])
  Phase 2:  outT[:, seg_s] = w2h[s]^T @ hT   (partial over the hidden half)
Host: out[token] += top_w * (outT_half0 + outT_half1)[:, col].T; + gates@b2.

Perf notes (measured on this hw):
  - The PE sustains only ~1.9-2.0 GHz with all 8 cores busy (power
    envelope; 2.4 GHz solo).  The kernel is within ~5% of that sustained
    roofline; remaining overhead is ~18ns per weight-group intercept.
  - Both weight matrices are SBUF-resident (16MB); hT is staged per
    segment so everything fits.  Steady-state DMA is xg in + outT (bf16)
    out only.
  - The Tile scheduler attaches a semaphore increment to EVERY matmul;
    _strip_pe_incs drops the unreferenced ones and remaps thresholds.
  - Legalization emits one LDWEIGHTS per matmul; redundant reloads of the
    same stationary tile are stripped post-compile (neutral on HW since
    LDW is hidden, but shrinks the instruction stream).
"""

import os
import sys

import numpy as np
import ml_dtypes

try:
    import concourse  # noqa: F401
except ImportError:  # fall back to the in-container repo checkout
    for _p in ("/opt/trn_rl_repo", os.path.expanduser("~/.axon_site/_ro/trn_rl_repo")):
        if os.path.isdir(_p) and _p not in sys.path:
            sys.path.insert(0, _p)

import concourse.mybir as mybir
import concourse.tile as tile
from concourse import bacc
from concourse.bass_utils import run_bass_kernel_spmd

D_MODEL, D_HID, N_EXPERTS, TOP_K = 1024, 4096, 8, 2
N_CORES = 8
P = 128
HO2 = D_HID // 2 // P  # 16 h-blocks per hidden half
DO = D_MODEL // P      # 8 d-blocks

BF16 = mybir.dt.bfloat16
F32 = mybir.dt.float32

_program_cache: dict[tuple, object] = {}
_weights_cache: dict = {}


# --------------------------------------------------------------------------
# Semaphore-increment stripping (see module docstring).
# --------------------------------------------------------------------------
def _strip_pe_incs(nc):
    fn = nc.m.functions[0]
    blocks = list(fn.blocks)

    def is_pe(name):
        return name.startswith("PE")

    block_incs = {}
    for bi, b in enumerate(blocks):
        incs = []
        for i in b.instructions:
            si = i.sync_info
            if si is None:
                continue
            for u in si.on_update:
                if (u.sync_type == "semaphore" and is_pe(u.ant_name)
                        and u.update_mode == "sem-inc"):
                    incs.append(i)
        if incs:
            block_incs[bi] = incs
    if len(block_incs) != 1:
        return 0
    (_, incs), = block_incs.items()
    n = len(incs)

    refs, wait_sites, upd_sites = set(), [], []
    for b in blocks:
        for i in b.instructions:
            si = i.sync_info
            if si is None:
                continue
            for w in si.on_wait:
                if (w.sync_type == "semaphore" and is_pe(w.ant_name)
                        and w.wait_mode in ("sem-ge-imm", "sem-eq-imm")
                        and w.wait_value is not None and 1 <= w.wait_value <= n):
                    refs.add(w.wait_value)
                    wait_sites.append((i, w))
            for u in si.on_update:
                if (u.sync_type == "semaphore" and is_pe(u.ant_name)
                        and u.update_mode != "sem-inc"
                        and u.update_value is not None
                        and 1 <= u.update_value <= n):
                    upd_sites.append((i, u))

    refs.add(n)
    keep = [False] * (n + 1)
    for v in refs:
        keep[v] = True
    for pos, inst in enumerate(incs, start=1):
        if "Matmult" not in type(inst).__name__:
            keep[pos] = True
    new_at, cnt = [0] * (n + 1), 0
    for p in range(1, n + 1):
        if keep[p]:
            cnt += 1
        new_at[p] = cnt

    dropped = 0
    for pos, inst in enumerate(incs, start=1):
        if keep[pos]:
            continue
        si = inst.sync_info
        inst.sync_info = mybir.SyncInfo(
            on_wait=list(si.on_wait),
            on_update=[u for u in si.on_update
                       if not (u.sync_type == "semaphore" and is_pe(u.ant_name)
                               and u.update_mode == "sem-inc")],
        )
        dropped += 1

    for wi, w in wait_sites:
        v, nv = w.wait_value, new_at[w.wait_value]
        if nv == v:
            continue
        si = wi.sync_info
        wi.sync_info = mybir.SyncInfo(
            on_wait=[
                mybir.SyncWait(sync_type=x.sync_type, id=x.id, ant_name=x.ant_name,
                               wait_mode=x.wait_mode, wait_value=nv,
                               wait_reg=x.wait_reg)
                if (x.sync_type == "semaphore" and is_pe(x.ant_name)
                    and x.wait_mode == w.wait_mode and x.wait_value == v)
                else x
                for x in si.on_wait
            ],
            on_update=list(si.on_update),
        )

    for ui, u in upd_sites:
        v, nv = u.update_value, new_at[u.update_value]
        if nv == v:
            continue
        si = ui.sync_info
        ui.sync_info = mybir.SyncInfo(
            on_wait=list(si.on_wait),
            on_update=[
                mybir.SyncUpdate(sync_type=x.sync_type, id=x.id,
                                 ant_name=x.ant_name, update_mode=x.update_mode,
                                 update_value=nv, update_reg=x.update_reg)
                if (x.sync_type == "semaphore" and is_pe(x.ant_name)
                    and x.update_mode == u.update_mode and x.update_value == v)
                else x
                for x in si.on_update
            ],
        )
    return dropped


# --------------------------------------------------------------------------
# Redundant-LDWEIGHTS stripping: legalization inserts one InstLdweights per
# InstMatmult even when consecutive matmuls use the identical stationary
# tile.  The array keeps its weights until the next LDWEIGHTS, so a reload
# of the exact same AP is a no-op -- drop it.  Tracker resets per block (a
# loop body's first LDW follows the previous iteration's LAST LDW, so
# first occurrences are always kept).  LDWs carrying sync are kept.
# --------------------------------------------------------------------------
def _strip_redundant_ldweights(nc):
    removed = 0
    for b in nc.m.functions[0].blocks:
        prev_key = None
        keep = []
        for inst in b.instructions:
            if isinstance(inst, mybir.InstLdweights):
                key = str(inst.ins[0])
                si = inst.sync_info
                has_sync = si is not None and (
                    len(si.on_wait) > 0 or len(si.on_update) > 0
                )
                if key == prev_key and not has_sync:
                    removed += 1
                    continue
                prev_key = key
            keep.append(inst)
        if removed:
            b.instructions[:] = keep
    return removed


def _n_tiles(C: int):
    """Split C columns into PSUM-bank-sized (<=512 f32) n-tiles."""
    tiles, off = [], 0
    while off < C:
        w = min(512, C - off)
        tiles.append((off, w))
        off += w
    return tiles


def _build_program(CA: int, CB: int = 0, repeat: int = 1, staggered: bool = False,
                   sem_strip: bool = True):
    """Two-segment program: segment A (cols [0,CA), expert A, hidden half)
    then segment B (cols [CA,CA+CB), expert B).  CB=0 -> single segment."""
    from concourse.bass import ds

    if staggered:
        # The staggered-reset sem protocol is incompatible with the PE-inc
        # strip (measured: deadlock); staggered alone wins anyway.
        sem_strip = False

    segments = [(0, 0, CA)]
    if CB:
        segments.append((1, CA, CB))
    C = CA + CB
    NSEG = len(segments)
    hmax = max(cw for _, _, cw in segments)

    nc = bacc.Bacc(
        "TRN2",
        target_bir_lowering=False,
        debug=False,
        num_devices=N_CORES,
    )
    xgT = nc.dram_tensor("xgT", [D_MODEL, C], BF16, kind="ExternalInput").ap()
    # host-packed: w1p[ki, s, m, ko, f] = w1half_s[ko*128+ki, m*128+f]
    w1 = nc.dram_tensor("w1", [P, NSEG, HO2, DO, P], BF16,
                        kind="ExternalInput").ap()
    # host-packed: w2p[ki, s, ko, d] = w2half_s[ko*128+ki, d]
    w2 = nc.dram_tensor("w2", [P, NSEG, HO2, D_MODEL], BF16,
                        kind="ExternalInput").ap()
    b1 = nc.dram_tensor("b1", [P, NSEG, HO2], F32, kind="ExternalInput").ap()
    outT = nc.dram_tensor("outT", [D_MODEL, C], BF16, kind="ExternalOutput").ap()

    xgT_r = xgT.rearrange("(po pi) f -> pi po f", pi=P)    # [128, 8, C]
    outT_r = outT.rearrange("(po pi) f -> pi po f", pi=P)  # [128, 8, C]

    with tile.TileContext(nc) as tc:
        with (
            tc.tile_pool(name="const", bufs=1) as const_pool,
            tc.tile_pool(name="ht_res", bufs=1) as ht_pool,
            tc.tile_pool(name="xg_res", bufs=1) as xg_pool,
            tc.tile_pool(name="w1_res", bufs=1) as w1_pool,
            tc.tile_pool(name="w2_res", bufs=1) as w2_pool,
            tc.tile_pool(name="p2_out", bufs=3) as out_pool,
            tc.tile_pool(name="psum", bufs=2, space="PSUM") as pp,
        ):
            b1_sb = const_pool.tile([P, NSEG, HO2], F32)
            nc.sync.dma_start(b1_sb[:], b1[:])

            # hT staged per segment so both resident weight matrices fit.
            hT_sb = ht_pool.tile([P, HO2, hmax], BF16)
            xg_sb = xg_pool.tile([P, DO, C], BF16)
            w1_sb = w1_pool.tile([P, NSEG, HO2, DO, P], BF16)
            w2_sb = w2_pool.tile([P, NSEG, HO2, D_MODEL], BF16)

            # Weights are iteration-invariant: prefetch once, outside the
            # loop body, in chunks on the gpsimd SWDGE queue.
            for s in range(NSEG):
                for j in range(HO2):
                    nc.gpsimd.dma_start(w1_sb[:, s, ds(j, 1)], w1[:, s, ds(j, 1)])
                    nc.gpsimd.dma_start(w2_sb[:, s, ds(j, 1)], w2[:, s, ds(j, 1)])

            def body():
                nc.sync.dma_start(xg_sb[:, 0], xgT_r[:, 0])
                nc.sync.dma_start(xg_sb[:, 1], xgT_r[:, 1])
                for j in range(2, DO):
                    nc.scalar.dma_start(xg_sb[:, j], xgT_r[:, j])

                for s, coff, cw in segments:
                    NT = _n_tiles(cw)
                    # ---------- Phase 1 (segment s, hidden half) ----------
                    for m in range(HO2):
                        psums = [
                            pp.tile([P, w], F32, name=f"p1_{s}_{m}_{i}",
                                    tag=f"ps_{i}")
                            for i, (toff, w) in enumerate(NT)
                        ]
                        for k in range(DO):
                            for i, (toff, w) in enumerate(NT):
                                nc.tensor.matmul(
                                    psums[i][:],
                                    w1_sb[:, s, m, k],
                                    xg_sb[:, k, ds(coff + toff, w)],
                                    start=(k == 0),
                                    stop=(k == DO - 1),
                                )
                        for i, (toff, w) in enumerate(NT):
                            nc.scalar.activation(
                                hT_sb[:, m, ds(toff, w)],
                                psums[i][:],
                                mybir.ActivationFunctionType.Gelu,
                                bias=b1_sb[:, s, ds(m, 1)],
                            )

                    # ---------- Phase 2 (segment s, partial out) ----------
                    for m in range(DO):
                        psums = [
                            pp.tile([P, w], F32, name=f"p2_{s}_{m}_{i}",
                                    tag=f"ps_{i}")
                            for i, (toff, w) in enumerate(NT)
                        ]
                        for k in range(HO2):
                            for i, (toff, w) in enumerate(NT):
                                nc.tensor.matmul(
                                    psums[i][:],
                                    w2_sb[:, s, k, ds(m * P, P)],
                                    hT_sb[:, k, ds(toff, w)],
                                    start=(k == 0),
                                    stop=(k == HO2 - 1),
                                )
                        ot = out_pool.tile([P, cw], BF16, tag="ot")
                        for i, (toff, w) in enumerate(NT):
                            nc.vector.tensor_copy(
                                out=ot[:, ds(toff, w)], in_=psums[i][:])
                        # Round-robin outT across three DMA queues: at
                        # 4.3MB/iter a single queue is near saturation and
                        # stalls the out-pool rotation (-> DVE -> PSUM ->
                        # PE).
                        q = (nc.sync, nc.gpsimd)[m % 2]
                        q.dma_start(outT_r[:, m, ds(coff, cw)], ot[:])

            if repeat > 1:
                hints = tuple(
                    getattr(mybir.EngineType, e)
                    for e in ("PE", "SP", "Activation", "DVE", "Pool")
                    if hasattr(mybir.EngineType, e)
                )
                with tc.For_i(
                    0, repeat, 1, hint_engines=hints, staggered_reset=staggered
                ):
                    body()
            else:
                body()

    if sem_strip:
        _strip_pe_incs(nc)
    nc.compile()
    _strip_redundant_ldweights(nc)
    return nc


def _route(x, gate_w):
    """Host router: softmax + top-2 + renormalize. Returns dispatch lists."""
    xf = np.ascontiguousarray(np.asarray(x, dtype=np.float32)).reshape(-1, D_MODEL)
    n_tok = xf.shape[0]
    gw = np.asarray(gate_w, dtype=np.float32)
    logits = xf @ gw.T  # [N, E]
    m = logits.max(axis=-1, keepdims=True)
    e = np.exp(logits - m, dtype=np.float32)
    scores = e / e.sum(axis=-1, keepdims=True)
    top_i = np.argpartition(-scores, TOP_K - 1, axis=-1)[:, :TOP_K]  # [N, K]
    top_w = np.take_along_axis(scores, top_i, axis=-1)
    top_w = top_w / top_w.sum(axis=-1, keepdims=True)
    idx_per_e, w_per_e = [], []
    for ex in range(N_EXPERTS):
        tok, slot = np.nonzero(top_i == ex)
        idx_per_e.append(tok)
        w_per_e.append(top_w[tok, slot])
    return xf, n_tok, scores, idx_per_e, w_per_e


def _pairing(idx_per_e):
    """Pair experts (largest count with smallest).  Returns
    (pairs, CA, CB): pairs[p] = (eA, eB) with count(eA) <= count(eB);
    CA/CB = padded max over pairs of the A/B segment widths."""
    counts = np.array([len(ix) for ix in idx_per_e])
    order = np.argsort(counts, kind="stable")
    pairs = [(int(order[p]), int(order[N_EXPERTS - 1 - p]))
             for p in range(N_EXPERTS // 2)]
    pad = lambda c: max(16, (int(c) + 3) // 4 * 4)  # noqa: E731
    CA = max(pad(counts[a]) for a, _ in pairs)
    CB = max(pad(counts[b]) for _, b in pairs)
    return pairs, int(CA), int(CB)


def _pack_weights(w1, w2, b1, pairs):
    """Cast + pre-pack per-core weight halves for the device layout."""
    wkey = (id(w1), id(w2), tuple(x for p in pairs for x in p))
    cached = _weights_cache.get(wkey)
    if cached is None:
        w1f = np.asarray(w1, dtype=np.float32)
        w2f = np.asarray(w2, dtype=np.float32)
        b1f = np.asarray(b1, dtype=np.float32).reshape(N_EXPERTS, D_HID)
        H2 = D_HID // 2
        w1p, w2p, b1p = [], [], []
        for p, (eA, eB) in enumerate(pairs):
            for h in range(2):
                r0, r1 = h * H2, (h + 1) * H2
                w1c = np.empty((P, 2, HO2, DO, P), dtype=ml_dtypes.bfloat16)
                w2c = np.empty((P, 2, HO2, D_MODEL), dtype=ml_dtypes.bfloat16)
                b1c = np.empty((P, 2, HO2), dtype=np.float32)
                for s, e in enumerate((eA, eB)):
                    a = w1f[e][:, r0:r1].astype(ml_dtypes.bfloat16)  # [1024, 2048]
                    w1c[:, s] = a.reshape(DO, P, HO2, P).transpose(1, 2, 0, 3)
                    b = w2f[e][r0:r1].astype(ml_dtypes.bfloat16)     # [2048, 1024]
                    w2c[:, s] = b.reshape(HO2, P, D_MODEL).transpose(1, 0, 2)
                    b1c[:, s] = b1f[e][r0:r1].reshape(HO2, P).T
                w1p.append(np.ascontiguousarray(w1c))
                w2p.append(np.ascontiguousarray(w2c))
                b1p.append(np.ascontiguousarray(b1c))
        cached = (w1p, w2p, b1p)
        _weights_cache.clear()
        _weights_cache[wkey] = cached
    return cached


def _make_in_maps(xf, idx_per_e, pairs, CA, CB, w1p, w2p, b1p):
    in_maps = []
    for p, (eA, eB) in enumerate(pairs):
        xgT = np.zeros((D_MODEL, CA + CB), dtype=ml_dtypes.bfloat16)
        ixA, ixB = idx_per_e[eA], idx_per_e[eB]
        xgT[:, :len(ixA)] = xf[ixA].T.astype(ml_dtypes.bfloat16)
        xgT[:, CA:CA + len(ixB)] = xf[ixB].T.astype(ml_dtypes.bfloat16)
        for h in range(2):
            core = 2 * p + h
            in_maps.append({
                "xgT": xgT,
                "w1": w1p[core],
                "w2": w2p[core],
                "b1": b1p[core],
            })
    return in_maps


def _run_device(x, gate_w, w1, b1, w2, b2):
    xf, n_tok, _scores, idx_per_e, w_per_e = _route(x, gate_w)
    pairs, CA, CB = _pairing(idx_per_e)

    key = (CA, CB)
    if key not in _program_cache:
        _program_cache[key] = _build_program(CA, CB)
    nc = _program_cache[key]

    w1p, w2p, b1p = _pack_weights(w1, w2, b1, pairs)
    in_maps = _make_in_maps(xf, idx_per_e, pairs, CA, CB, w1p, w2p, b1p)

    res = run_bass_kernel_spmd(nc, in_maps, core_ids=list(range(N_CORES)))

    out_full = np.zeros((n_tok, D_MODEL), dtype=np.float32)
    for p, (eA, eB) in enumerate(pairs):
        acc = (np.asarray(res.results[2 * p]["outT"]).astype(np.float32)
               + np.asarray(res.results[2 * p + 1]["outT"]).astype(np.float32))
        for s, e in enumerate((eA, eB)):
            ix = idx_per_e[e]
            off = 0 if s == 0 else CA
            out_full[ix] += w_per_e[e][:, None] * acc[:, off:off + len(ix)].T
    gates = np.zeros((n_tok, N_EXPERTS), dtype=np.float32)
    for ex in range(N_EXPERTS):
        gates[idx_per_e[ex], ex] = w_per_e[ex]
    out_full += gates @ np.asarray(b2, dtype=np.float32).reshape(N_EXPERTS, D_MODEL)
    return out_full, res


def kernel(x, gate_w, w1, b1, w2, b2):
    out_full, _res = _run_device(x, gate_w, w1, b1, w2, b2)
    B, T, _ = np.asarray(x).shape
    return out_full.reshape(B, T, D_MODEL)


def _bench_maps(inputs):
    """(in_maps, (CA, CB)) for test.py's repeat-loop timing harness."""
    xf, n_tok, _s, idx_per_e, w_per_e = _route(inputs["x"], inputs["gate_w"])
    pairs, CA, CB = _pairing(idx_per_e)
    w1p, w2p, b1p = _pack_weights(inputs["w1"], inputs["w2"], inputs["b1"], pairs)
    in_maps = _make_in_maps(xf, idx_per_e, pairs, CA, CB, w1p, w2p, b1p)
    return in_maps, (CA, CB)
